# revision 42
# baseline (speedup 1.0000x reference)
"""Trainium2 Bass kernel for a dense transformer decoder block.

Hybrid sharding over 8 NeuronCores:
 - norm+rope run once per token (sequence-parallel) and the transposed
   activations are AllGathered in two token-half chunks (bf16);
 - attention is head-sharded (2 heads/core) with an AllToAll, split in
   two token-halves so the second half overlaps the WO matmuls;
 - the FFN is sequence-parallel: each core runs the full 8192-wide FFN
   for its own 1024 tokens, streaming all FFN weights from HBM under
   the matmuls -- no AllGather/ReduceScatter at all.
Matmul operands are bf16 (cheapest self-loading weight path, N=512 ISA
cap); accumulation, softmax statistics and residuals stay fp32.
"""
import sys

if '/opt/trn_rl_repo' not in sys.path:
    sys.path.insert(0, '/opt/trn_rl_repo')

import numpy as np
import ml_dtypes
from contextlib import ExitStack

BF = ml_dtypes.bfloat16

B, S, E, H, DH, FF = 4, 2048, 2048, 16, 128, 8192
P = 128
NCORES = 8
HLOC = H // NCORES          # 2 heads per core
FLOC = FF // NCORES         # 1024 FFN cols per core
TOK = B * S                 # 8192 tokens
TSL = TOK // NCORES         # 1024-token slice per core
EC = E // P                 # 16 chunks of the embedding dim
GSUB = FLOC // P            # 8 F-subtiles per core
EPS = 1e-5
ATB = 256                   # stage-A token block
NTB = S // ATB              # 8 blocks per batch

_CACHE = {}


def _build():
    import concourse.bacc as bacc
    import concourse.mybir as mybir
    import concourse.tile as tile
    import concourse.tile_utils as tile_utils
    from concourse.masks import make_identity

    tile_utils.max_sbuf_usage = 204 * 1024

    F32 = mybir.dt.float32
    F32R = mybir.dt.float32r
    BF16 = mybir.dt.bfloat16
    AF = mybir.ActivationFunctionType
    OP = mybir.AluOpType

    nc = bacc.Bacc(None, target_bir_lowering=False)
    names = {}

    with tile.TileContext(nc) as tc:
        with tc.tile_pool(name="dram", bufs=1, space="DRAM") as dram:
            # ---- external inputs (per-core host-prepared) ----
            xsl_in = dram.tile([TSL, E], F32, kind="ExternalInput")
            wqkv_in = dram.tile([EC, P, 6 * P], BF16, kind="ExternalInput")
            wo_in = dram.tile([H, P, E], BF16, kind="ExternalInput")
            # FFN weights, sequence-parallel: full F dim streamed per core.
            # wgl: [fb=64][p=128][ec=16 x (gate 128 | lin 128)]
            wgl_in = dram.tile([FF // P, P, EC * 2 * P], BF16,
                               kind="ExternalInput")
            # wout: [fs=16][ecol=4][fb=4 x p=128][c=512]
            wout_in = dram.tile([16, 4, 4 * P, 512], BF16,
                                kind="ExternalInput")
            cos_in = dram.tile([TSL, 64], F32, kind="ExternalInput")
            sin_in = dram.tile([TSL, 64], F32, kind="ExternalInput")
            mask_in = dram.tile([4, P, 512], BF16, kind="ExternalInput")
            onec_in = dram.tile([P, 1], F32R, kind="ExternalInput")
            oner_in = dram.tile([1, P], F32R, kind="ExternalInput")
            out_sl = dram.tile([TSL, E], F32, kind="ExternalOutput")
            names.update(
                xsl=xsl_in.name, wqkv=wqkv_in.name, wo=wo_in.name,
                wgl=wgl_in.name, wout=wout_in.name, cos=cos_in.name,
                sin=sin_in.name, mask=mask_in.name, onec=onec_in.name,
                oner=oner_in.name, out=out_sl.name)

            # ---- internal DRAM (collective bounce) ----
            # yT AllGather in 2 token-halves: own [E, 512] -> all [8*E, 512]
            ag_in = [dram.tile([E, 4 * P], BF16, name=f"ag_in{i}")
                     for i in range(2)]
            ag_out = [dram.tile([NCORES * E, 4 * P], BF16, name=f"ag_out{i}",
                                addr_space="Shared")
                      for i in range(2)]
            # AllToAll split in 2 token-halves of each destination slice
            a2a_in = [dram.tile([NCORES * HLOC * P, TSL // 2], BF16,
                                name=f"a2a_in{i}") for i in range(2)]
            a2a_out = [dram.tile([NCORES * HLOC * P, TSL // 2], BF16,
                                 name=f"a2a_out{i}") for i in range(2)]

            RG = [list(range(NCORES))]

            with tc.tile_pool(name="cst", bufs=1) as cst:
                ident = cst.tile([P, P], F32)
                make_identity(nc, ident[:])
                eps_t = cst.tile([P, 1], F32)
                nc.gpsimd.memset(eps_t[:], EPS)

                # residual slice pool opened before stage A (LIFO) so it
                # survives into WO/FFN; DMAs emitted after A0
                x2_stack = ExitStack()
                x2_p = x2_stack.enter_context(tc.tile_pool(name="x2_p", bufs=1))

                # ================= stage A: norm+rope+QKV+attention =========
                stgA = ExitStack()
                wqkv_p = stgA.enter_context(tc.tile_pool(name="wqkv_p", bufs=1))
                tabs = stgA.enter_context(tc.tile_pool(name="tabs", bufs=1))

                wqkv_sb = wqkv_p.tile([P, EC * 6 * P], BF16)
                nc.sync.dma_start(
                    out=wqkv_sb[:].rearrange("p (e c) -> p e c", e=EC),
                    in_=wqkv_in[:].rearrange("e p c -> p e c"))
                ones_col = tabs.tile([P, 1], F32R)
                nc.sync.dma_start(out=ones_col[:], in_=onec_in[:])
                ones_row = tabs.tile([1, P], F32R)
                nc.sync.dma_start(out=ones_row[:], in_=oner_in[:])
                masks = tabs.tile([P, 4 * 512], BF16)
                nc.sync.dma_start(
                    out=masks[:].rearrange("p (m w) -> p m w", m=4),
                    in_=mask_in[:].rearrange("m p w -> p m w"))

                # ---- A0: norm+rope+transpose own slice, then AllGather -----
                a0 = ExitStack()
                a0_sb = a0.enter_context(tc.tile_pool(name="a0_sb", bufs=3))
                a0_ps = a0.enter_context(tc.tile_pool(name="a0_ps", bufs=2, space="PSUM"))

                cos_all = a0_sb.tile([P, 8 * 64], F32, tag="cos")
                sin_all = a0_sb.tile([P, 8 * 64], F32, tag="sin")
                nc.sync.dma_start(
                    out=cos_all[:].rearrange("p (r j) -> p r j", r=8),
                    in_=cos_in[:].rearrange("(r p) j -> p r j", p=P))
                nc.sync.dma_start(
                    out=sin_all[:].rearrange("p (r j) -> p r j", r=8),
                    in_=sin_in[:].rearrange("(r p) j -> p r j", p=P))

                for tt in range(8):
                    x_t = a0_sb.tile([P, E], F32, tag="x")
                    nc.sync.dma_start(out=x_t[:], in_=xsl_in[P * tt:P * (tt + 1), :])
                    scr = a0_sb.tile([P, E], F32, tag="y")
                    ssq = a0_sb.tile([P, 1], F32, tag="ssq")
                    nc.scalar.activation(scr[:], x_t[:], AF.Square,
                                         accum_out=ssq[:])
                    sq = a0_sb.tile([P, 1], F32, tag="sq")
                    nc.scalar.activation(sq[:], ssq[:], AF.Sqrt,
                                         scale=1.0 / E, bias=eps_t[:])
                    s_t = a0_sb.tile([P, 1], F32, tag="s")
                    nc.vector.reciprocal(s_t[:], sq[:])
                    # rope with rmsnorm scale folded in:
                    #   y1 = (x1*s)*cos - (x2*s)*sin
                    #   y2 = (x2*s)*cos + (x1*s)*sin
                    y_t = a0_sb.tile([P, E], F32, tag="y")
                    t1 = a0_sb.tile([P, E], F32, tag="t1")
                    xr = x_t[:].rearrange("p (c two h) -> p c two h", two=2, h=64)
                    yr = y_t[:].rearrange("p (c two h) -> p c two h", two=2, h=64)
                    tr = t1[:].rearrange("p (c two h) -> p c two h", two=2, h=64)
                    cb = cos_all[:, 64 * tt:64 * (tt + 1)].rearrange(
                        "p (o j) -> p o j", o=1).broadcast_to([P, EC, 64])
                    sb_ = sin_all[:, 64 * tt:64 * (tt + 1)].rearrange(
                        "p (o j) -> p o j", o=1).broadcast_to([P, EC, 64])
                    nc.vector.scalar_tensor_tensor(
                        out=tr[:, :, 0], in0=xr[:, :, 1], scalar=s_t[:],
                        in1=sb_, op0=OP.mult, op1=OP.mult)
                    nc.vector.scalar_tensor_tensor(
                        out=yr[:, :, 0], in0=xr[:, :, 0], scalar=s_t[:],
                        in1=cb, op0=OP.mult, op1=OP.mult)
                    nc.vector.tensor_tensor(
                        out=yr[:, :, 0], in0=yr[:, :, 0], in1=tr[:, :, 0],
                        op=OP.subtract)
                    nc.vector.scalar_tensor_tensor(
                        out=tr[:, :, 1], in0=xr[:, :, 0], scalar=s_t[:],
                        in1=sb_, op0=OP.mult, op1=OP.mult)
                    nc.vector.scalar_tensor_tensor(
                        out=yr[:, :, 1], in0=xr[:, :, 1], scalar=s_t[:],
                        in1=cb, op0=OP.mult, op1=OP.mult)
                    nc.vector.tensor_tensor(
                        out=yr[:, :, 1], in0=yr[:, :, 1], in1=tr[:, :, 1],
                        op=OP.add)
                    # transpose 16 chunks, stage out to this token-half's
                    # AllGather input
                    yto = a0_sb.tile([P, EC * P], BF16, tag="yto")
                    yto_v = yto[:].rearrange("p (c t) -> p c t", t=P)
                    for gch in range(4):
                        tps = a0_ps.tile([P, 512], F32, tag="tps")
                        for c4 in range(4):
                            c = 4 * gch + c4
                            nc.tensor.transpose(
                                tps[:, P * c4:P * (c4 + 1)],
                                y_t[:, P * c:P * (c + 1)], ident[:])
                        nc.scalar.copy(
                            yto_v[:, 4 * gch:4 * gch + 4, :],
                            tps[:].rearrange("p (c t) -> p c t", t=P))
                    c = tt // 4
                    nc.sync.dma_start(
                        out=ag_in[c][:].rearrange("(ec p) t -> p ec t", p=P)[
                            :, :, P * (tt % 4):P * (tt % 4 + 1)],
                        in_=yto_v[:, :, :])
                    if tt % 4 == 3:
                        nc.gpsimd.collective_compute(
                            "AllGather", OP.bypass, replica_groups=RG,
                            ins=[ag_in[c][:]], outs=[ag_out[c][:]])
                a0.close()

                # QKV/attention pools open after A0's space is released
                ytb_p = stgA.enter_context(tc.tile_pool(name="ytb", bufs=4))
                qkvb_p = stgA.enter_context(tc.tile_pool(name="qkvb", bufs=2))
                st_sb = stgA.enter_context(tc.tile_pool(name="st_sb", bufs=2))
                at_sb = stgA.enter_context(tc.tile_pool(name="at_sb", bufs=3))
                st_ps = stgA.enter_context(tc.tile_pool(name="st_ps", bufs=1, space="PSUM"))
                qkv_ps = stgA.enter_context(tc.tile_pool(name="qkv_ps", bufs=3, space="PSUM"))
                at_s_ps = stgA.enter_context(tc.tile_pool(name="at_s_ps", bufs=2, space="PSUM"))
                at_o_ps = stgA.enter_context(tc.tile_pool(name="at_o_ps", bufs=1, space="PSUM"))
                at_db_ps = stgA.enter_context(tc.tile_pool(name="at_db_ps", bufs=1, space="PSUM"))

                # ---- QKV from gathered yT + attention, batch by batch ------
                SC = 1.0 / float(np.sqrt(DH))
                for b in range(B):
                    qt_b = qkvb_p.tile([P, HLOC * S], BF16, tag="qt")
                    kt_b = qkvb_p.tile([P, HLOC * S], BF16, tag="kt")
                    v_b = qkvb_p.tile([P, HLOC * S], BF16, tag="vb")
                    for h in range(2):          # 512-token halves of each rank
                        for rr in (2 * b, 2 * b + 1):
                            ygs = []
                            for ech in range(2):
                                yg = ytb_p.tile([P, 8 * 512], BF16, tag="yg",
                                                name=f"yg{ech}")
                                nc.sync.dma_start(
                                    out=yg[:].rearrange("p (e w) -> p e w",
                                                        e=8),
                                    in_=ag_out[h][
                                        E * rr + 1024 * ech:
                                        E * rr + 1024 * (ech + 1), :
                                    ].rearrange("(e p) t -> p e t", p=P))
                                ygs.append(yg)
                            for g in range(6):
                                pq = qkv_ps.tile([P, 512], F32, tag="pq")
                                for ec in range(EC):
                                    nc.tensor.matmul(
                                        pq[:],
                                        wqkv_sb[:, 6 * P * ec + P * g:
                                                6 * P * ec + P * (g + 1)],
                                        ygs[ec // 8][:, 512 * (ec % 8):
                                                     512 * (ec % 8 + 1)],
                                        start=(ec == 0), stop=(ec == EC - 1))
                                hl = g % 2
                                col = S * hl + 1024 * (rr % 2) + 512 * h
                                if g < 2:      # Q heads, scale by 1/sqrt(DH)
                                    nc.scalar.activation(
                                        qt_b[:, col:col + 512], pq[:], AF.Copy,
                                        scale=SC)
                                elif g < 4:    # K heads
                                    nc.scalar.copy(kt_b[:, col:col + 512], pq[:])
                                else:          # V heads -> transpose to [tok, DH]
                                    vt_tmp = st_sb.tile([P, 512], F32, tag="vt")
                                    nc.scalar.copy(vt_tmp[:], pq[:])
                                    tpv = st_ps.tile([P, 512], F32, tag="tps")
                                    for j in range(4):
                                        nc.tensor.transpose(
                                            tpv[:, P * j:P * (j + 1)],
                                            vt_tmp[:, P * j:P * (j + 1)], ident[:])
                                    nc.scalar.copy(v_b[:, col:col + 512], tpv[:])
                    # ---- attention for batch b, both local heads ----
                    # qb order: evens first so the first-half AllToAll can
                    # fire before the odd qbs of the last batch.
                    for h in range(HLOC):
                        hs = S * h
                        for qb in (0, 2, 1, 3):
                            nk = 4 * qb + 4
                            ps_o = at_o_ps.tile([P, 512], F32, tag="pso")
                            acc = [at_sb.tile([P, 512], F32R, tag="acc0",
                                              name="acc0"),
                                   at_sb.tile([P, 512], F32R, tag="acc1",
                                              name="acc1")]
                            for kt in range(nk):
                                ps_s = at_s_ps.tile([P, 512], F32, tag="pss")
                                nc.tensor.matmul(
                                    ps_s[:],
                                    kt_b[:, hs + P * kt: hs + P * (kt + 1)],
                                    qt_b[:, hs + 512 * qb: hs + 512 * (qb + 1)],
                                    start=True, stop=True)
                                probs = at_sb.tile([P, 512], BF16, tag="probs")
                                nc.scalar.activation(probs[:], ps_s[:], AF.Exp)
                                if kt >= 4 * qb:
                                    m = kt - 4 * qb
                                    nc.gpsimd.tensor_tensor(
                                        out=probs[:], in0=probs[:],
                                        in1=masks[:, 512 * m:512 * (m + 1)],
                                        op=OP.mult)
                                a_ = acc[kt % 2]
                                eng = nc.vector if kt % 2 == 0 else nc.gpsimd
                                if kt < 2:
                                    eng.tensor_copy(a_[:], probs[:])
                                else:
                                    eng.tensor_tensor(
                                        out=a_[:], in0=a_[:], in1=probs[:],
                                        op=OP.add)
                                nc.tensor.matmul(
                                    ps_o[:],
                                    v_b[:, hs + P * kt: hs + P * (kt + 1)],
                                    probs[:],
                                    start=(kt == 0), stop=(kt == nk - 1),
                                    skip_group_check=True)
                            ps_d = at_db_ps.tile([1, 512], F32, tag="db")
                            nc.tensor.matmul(ps_d[:], ones_col[:], acc[0][:],
                                             start=True, stop=False)
                            nc.tensor.matmul(ps_d[:], ones_col[:], acc[1][:],
                                             start=False, stop=True)
                            rd = at_sb.tile([1, 512], F32R, tag="rd")
                            with nc.allow_low_precision(reason="softmax denom"):
                                nc.vector.reciprocal(rd[:], ps_d[:])
                            ps_b = at_db_ps.tile([P, 512], F32, tag="db")
                            nc.tensor.matmul(ps_b[:], ones_row[:], rd[:],
                                             start=True, stop=True)
                            osb = at_sb.tile([P, 512], F32, tag="osb")
                            nc.scalar.copy(osb[:], ps_o[:])
                            ot = at_sb.tile([P, 512], BF16, tag="ot")
                            nc.vector.tensor_tensor(out=ot[:], in0=osb[:],
                                                    in1=ps_b[:], op=OP.mult)
                            dest = 2 * b + qb // 2
                            a2a_b = a2a_in[qb % 2]
                            a2a_v = a2a_b[:].rearrange("(d r) t -> d r t",
                                                       d=NCORES)
                            nc.sync.dma_start(
                                out=a2a_v[dest, P * h:P * (h + 1), :],
                                in_=ot[:])
                            if b == 3 and h == 1 and qb == 2:
                                # all even-qb outputs written: fire half 0
                                nc.gpsimd.collective_compute(
                                    "AllToAll", OP.bypass, replica_groups=RG,
                                    ins=[a2a_in[0][:]], outs=[a2a_out[0][:]])
                    if b == 0:
                        # residual slice prefetch: emitted once the AllGather
                        # window's DMA burst is over
                        x2_t = [None] * 8
                        for tt in range(8):
                            x2_t[tt] = x2_p.tile([P, E], F32, tag=f"x2_{tt}",
                                                 name=f"x2t{tt}")
                            nc.sync.dma_start(
                                out=x2_t[tt][:],
                                in_=xsl_in[P * tt:P * (tt + 1), :])
                stgA.close()

                nc.gpsimd.collective_compute(
                    "AllToAll", OP.bypass, replica_groups=RG,
                    ins=[a2a_in[1][:]], outs=[a2a_out[1][:]])

                # ================= WO phase (own token slice, 2 halves) =====
                # y2T/norm pools open first so norm2 of half 0 can run while
                # half 1 of WO is still waiting on its AllToAll
                y2T_stack = ExitStack()
                y2T_p = y2T_stack.enter_context(tc.tile_pool(name="y2T_p", bufs=1))
                n2_stack = ExitStack()
                n2_sb = n2_stack.enter_context(tc.tile_pool(name="n2_sb", bufs=2))
                n2_ps = n2_stack.enter_context(tc.tile_pool(name="n2_ps", bufs=2, space="PSUM"))
                y2T_sb = y2T_p.tile([P, EC * TSL], BF16, tag="y2T")

                def norm2_tt(tt):
                    scr2 = n2_sb.tile([P, E], F32, tag="y2", name="scr2")
                    ssq2 = n2_sb.tile([P, 1], F32, tag="ssq2", name="ssq2")
                    nc.scalar.activation(scr2[:], x2_t[tt][:], AF.Square,
                                         accum_out=ssq2[:])
                    sq2 = n2_sb.tile([P, 1], F32, tag="sq2", name="sq2")
                    nc.scalar.activation(sq2[:], ssq2[:], AF.Sqrt,
                                         scale=1.0 / E, bias=eps_t[:])
                    s2 = n2_sb.tile([P, 1], F32, tag="s2", name="s2")
                    nc.vector.reciprocal(s2[:], sq2[:])
                    y2_t = n2_sb.tile([P, E], F32, tag="y2", name="y2t")
                    nc.scalar.activation(y2_t[:], x2_t[tt][:], AF.Copy,
                                         scale=s2[:])
                    y2T_dst = y2T_sb[:].rearrange("p (c t) -> p c t", t=TSL)
                    for gch in range(4):
                        tps = n2_ps.tile([P, 512], F32, tag="tps", name="tps")
                        for c4 in range(4):
                            c = 4 * gch + c4
                            nc.tensor.transpose(
                                tps[:, P * c4:P * (c4 + 1)],
                                y2_t[:, P * c:P * (c + 1)], ident[:])
                        nc.scalar.copy(
                            y2T_dst[:, 4 * gch:4 * gch + 4, P * tt:P * (tt + 1)],
                            tps[:].rearrange("p (c t) -> p c t", t=P))

                wo_pools = ExitStack()
                ot_p = wo_pools.enter_context(tc.tile_pool(name="ot_p", bufs=2))
                wo_p = wo_pools.enter_context(tc.tile_pool(name="wo_p", bufs=2))
                wo_ps = wo_pools.enter_context(tc.tile_pool(name="wo_ps", bufs=2, space="PSUM"))

                for half in range(2):
                    # [2048 hd rows, 512 tok] -> sbuf [p, (hc, tok)]
                    ot_h = ot_p.tile([P, H * 512], BF16, tag="oth")
                    nc.sync.dma_start(
                        out=ot_h[:].rearrange("p (hc t) -> p hc t", hc=H),
                        in_=a2a_out[half][:].rearrange("(hc p) t -> p hc t",
                                                       p=P))
                    for ecol in range(4):
                        wo_c = wo_p.tile([P, H * 512], BF16, tag="woc")
                        nc.sync.dma_start(
                            out=wo_c[:].rearrange("p (hc w) -> p hc w", hc=H),
                            in_=wo_in[:, :, 512 * ecol:512 * (ecol + 1)].rearrange(
                                "hc p w -> p hc w"))
                        for tloc in range(4):
                            tt = 4 * half + tloc
                            ps = wo_ps.tile([P, 512], F32, tag="ps")
                            for hc in range(H):
                                nc.tensor.matmul(
                                    ps[:],
                                    ot_h[:, 512 * hc + P * tloc:
                                         512 * hc + P * (tloc + 1)],
                                    wo_c[:, 512 * hc:512 * (hc + 1)],
                                    start=(hc == 0), stop=(hc == H - 1))
                            nc.vector.tensor_tensor(
                                out=x2_t[tt][:, 512 * ecol:512 * (ecol + 1)],
                                in0=ps[:],
                                in1=x2_t[tt][:, 512 * ecol:512 * (ecol + 1)],
                                op=OP.add)
                    for tloc in range(4):
                        norm2_tt(4 * half + tloc)
                wo_pools.close()
                n2_stack.close()

                # ===== FFN, sequence-parallel: full F dim over own tokens ====
                ffn = ExitStack()
                wgl_p = ffn.enter_context(tc.tile_pool(name="wgl_p", bufs=2))
                wout_p = ffn.enter_context(tc.tile_pool(name="wout_p", bufs=2))
                h_p = ffn.enter_context(tc.tile_pool(name="h_p", bufs=4))
                f1_sb = ffn.enter_context(tc.tile_pool(name="f1_sb", bufs=2))
                f1_ps = ffn.enter_context(tc.tile_pool(name="f1_ps", bufs=1, space="PSUM"))
                f2_ps = ffn.enter_context(tc.tile_pool(name="f2_ps", bufs=2, space="PSUM"))

                NFS = 16                 # f-super blocks of 512
                FBP = 4                  # 128-wide f blocks per super
                for fs in range(NFS):
                    h_t = [None] * FBP
                    for fb in range(FBP):
                        gfb = FBP * fs + fb
                        wgl_sb = wgl_p.tile([P, EC * 2 * P], BF16, tag="wgl")
                        nc.sync.dma_start(out=wgl_sb[:], in_=wgl_in[gfb])
                        ps_g = [None] * 2
                        ps_l = [None] * 2
                        for hf in range(2):
                            ps_g[hf] = f1_ps.tile([P, 512], F32, tag=f"psg{hf}", name=f"psg{hf}")
                            ps_l[hf] = f1_ps.tile([P, 512], F32, tag=f"psl{hf}", name=f"psl{hf}")
                            for ec in range(EC):
                                wb = 2 * P * ec
                                nc.tensor.matmul(
                                    ps_g[hf][:],
                                    wgl_sb[:, wb:wb + P],
                                    y2T_sb[:, TSL * ec + 512 * hf: TSL * ec + 512 * (hf + 1)],
                                    start=(ec == 0), stop=(ec == EC - 1),
                                    skip_group_check=True)
                                nc.tensor.matmul(
                                    ps_l[hf][:],
                                    wgl_sb[:, wb + P:wb + 2 * P],
                                    y2T_sb[:, TSL * ec + 512 * hf: TSL * ec + 512 * (hf + 1)],
                                    start=(ec == 0), stop=(ec == EC - 1),
                                    skip_group_check=True)
                        h_t[fb] = h_p.tile([P, TSL], BF16, tag="h",
                                           name=f"h_{fs}_{fb}")
                        for hf in range(2):
                            tmp_g = f1_sb.tile([P, 512], F32, tag="tmpg")
                            nc.scalar.activation(tmp_g[:], ps_g[hf][:], AF.Gelu)
                            nc.vector.tensor_tensor(
                                out=h_t[fb][:, 512 * hf:512 * (hf + 1)],
                                in0=tmp_g[:], in1=ps_l[hf][:], op=OP.mult)
                    # F2: accumulate this f-super's contribution into x2 acc
                    for ecol in range(4):
                        wout_sb = wout_p.tile([P, FBP * 512], BF16, tag="wout")
                        nc.sync.dma_start(
                            out=wout_sb[:].rearrange("p (b w) -> p b w", b=FBP),
                            in_=wout_in[fs, ecol].rearrange(
                                "(b p) w -> p b w", p=P))
                        for tt in range(8):
                            ps_o = f2_ps.tile([P, 512], F32, tag="pso")
                            for fb in range(FBP):
                                nc.tensor.matmul(
                                    ps_o[:],
                                    h_t[fb][:, P * tt:P * (tt + 1)],
                                    wout_sb[:, 512 * fb:512 * (fb + 1)],
                                    start=(fb == 0), stop=(fb == FBP - 1))
                            nc.vector.tensor_tensor(
                                out=x2_t[tt][:, 512 * ecol:512 * (ecol + 1)],
                                in0=ps_o[:],
                                in1=x2_t[tt][:, 512 * ecol:512 * (ecol + 1)],
                                op=OP.add)
                ffn.close()
                y2T_stack.close()
                for tt in range(8):
                    nc.sync.dma_start(out=out_sl[P * tt:P * (tt + 1), :],
                                      in_=x2_t[tt][:])
                x2_stack.close()
    nc.compile()
    return nc, names


def _prep_shared(inputs):
    """Host-side prep of tensors identical on every core."""
    x = np.ascontiguousarray(
        np.asarray(inputs["inputs"], np.float32).reshape(TOK, E))
    wo = np.asarray(inputs["wo"], np.float32)
    w_gate = np.asarray(inputs["w_gate"], np.float32)
    w_lin = np.asarray(inputs["w_lin"], np.float32)
    w_out = np.asarray(inputs["w_out"], np.float32)
    gamma_attn = np.asarray(inputs["gamma_attn"], np.float32)
    gamma_ffn = np.asarray(inputs["gamma_ffn"], np.float32)
    positions = np.asarray(inputs["positions"])

    wo_h = np.ascontiguousarray(wo.reshape(H, P, E).astype(BF))
    # wgl: [fb=64, p=128, ec=16, gate 128 | lin 128] with gamma_ffn folded
    wg4 = (w_gate * gamma_ffn[:, None]).reshape(EC, P, FF // P, P)
    wl4 = (w_lin * gamma_ffn[:, None]).reshape(EC, P, FF // P, P)
    wgl_h = np.empty((FF // P, P, EC, 2 * P), np.float32)
    wgl_h[..., :P] = wg4.transpose(2, 1, 0, 3)
    wgl_h[..., P:] = wl4.transpose(2, 1, 0, 3)
    wgl_h = np.ascontiguousarray(
        wgl_h.reshape(FF // P, P, EC * 2 * P).astype(BF))
    # wout: [fs=16, ecol=4, fb*p=512, c=512]
    wout_h = np.ascontiguousarray(
        w_out.reshape(16, 4, P, 4, 512).transpose(0, 3, 1, 2, 4)
        .reshape(16, 4, 4 * P, 512).astype(BF))

    # rope tables (gamma_attn folded; reference gamma is all-ones so it is
    # uniform -- assert so silent wrong answers are impossible)
    assert np.all(gamma_attn == gamma_attn[0]), \
        "non-uniform gamma_attn needs full-width rope tables"
    half = DH // 2
    inv_freq = (1.0 / (10000.0 ** (np.arange(half, dtype=np.float32) / half))
                ).astype(np.float32)
    ang = positions.astype(np.float32)[:, None] * inv_freq[None, :]
    g0 = float(gamma_attn[0])
    cos = (np.cos(ang) * g0).astype(np.float32)
    sin = (np.sin(ang) * g0).astype(np.float32)

    k_i = np.arange(P)[:, None]
    q_i = np.arange(512)[None, :]
    msk = np.stack([(P * m + k_i <= q_i) for m in range(4)]).astype(BF)
    return {
        "_x": x, "_cos": cos, "_sin": sin,
        "wo": wo_h, "wgl": wgl_h, "wout": wout_h, "mask": msk,
        "onec": np.ones((P, 1), np.float32),
        "oner": np.ones((1, P), np.float32),
    }


def _prep_inputs(inputs, r, shared):
    """Per-core host-side input prep for core r."""
    wq = np.asarray(inputs["wq"], np.float32)
    wk = np.asarray(inputs["wk"], np.float32)
    wv = np.asarray(inputs["wv"], np.float32)
    h0 = HLOC * r

    def _slice_qkv(w):   # [E, H, DH] -> [EC, P, HLOC*DH]
        return w[:, h0:h0 + HLOC, :].reshape(EC, P, HLOC * DH)

    wqkv = np.concatenate([_slice_qkv(wq), _slice_qkv(wk), _slice_qkv(wv)],
                          axis=2)
    d = {k: v for k, v in shared.items() if not k.startswith("_")}
    d["xsl"] = np.ascontiguousarray(shared["_x"][TSL * r:TSL * (r + 1)])
    sl = slice(1024 * (r % 2), 1024 * (r % 2) + TSL)
    d["cos"] = np.ascontiguousarray(shared["_cos"][sl])
    d["sin"] = np.ascontiguousarray(shared["_sin"][sl])
    d["wqkv"] = np.ascontiguousarray(wqkv.astype(BF))
    return d


def kernel(**inputs) -> np.ndarray:
    from concourse.bass_utils import run_bass_kernel_spmd

    if "nc" not in _CACHE:
        _CACHE["nc"], _CACHE["names"] = _build()
    nc, names = _CACHE["nc"], _CACHE["names"]

    shared = _prep_shared(inputs)
    in_maps = []
    for r in range(NCORES):
        prep = _prep_inputs(inputs, r, shared)
        in_maps.append({names[k]: v for k, v in prep.items()})

    res = run_bass_kernel_spmd(nc, in_maps, core_ids=list(range(NCORES)))
    out = np.empty((TOK, E), np.float32)
    for r in range(NCORES):
        out[TSL * r:TSL * (r + 1)] = res.results[r][names["out"]]
    return out.reshape(B, S, E)



# revision 48
# speedup vs baseline: 1.0146x; 1.0146x over previous
"""Trainium2 Bass kernel for a dense transformer decoder block.

Hybrid sharding over 8 NeuronCores:
 - norm+rope run once per token (sequence-parallel) and the transposed
   activations are AllGathered in two token-half chunks (bf16);
 - attention is head-sharded (2 heads/core) with an AllToAll, split in
   two token-halves so the second half overlaps the WO matmuls;
 - the FFN is sequence-parallel: each core runs the full 8192-wide FFN
   for its own 1024 tokens, streaming all FFN weights from HBM under
   the matmuls -- no AllGather/ReduceScatter at all.
Matmul operands are bf16 (cheapest self-loading weight path, N=512 ISA
cap); accumulation, softmax statistics and residuals stay fp32.
"""
import sys

if '/opt/trn_rl_repo' not in sys.path:
    sys.path.insert(0, '/opt/trn_rl_repo')

import numpy as np
import ml_dtypes
from contextlib import ExitStack

BF = ml_dtypes.bfloat16

B, S, E, H, DH, FF = 4, 2048, 2048, 16, 128, 8192
P = 128
NCORES = 8
HLOC = H // NCORES          # 2 heads per core
FLOC = FF // NCORES         # 1024 FFN cols per core
TOK = B * S                 # 8192 tokens
TSL = TOK // NCORES         # 1024-token slice per core
EC = E // P                 # 16 chunks of the embedding dim
GSUB = FLOC // P            # 8 F-subtiles per core
EPS = 1e-5
ATB = 256                   # stage-A token block
NTB = S // ATB              # 8 blocks per batch

_CACHE = {}


def _build():
    import concourse.bacc as bacc
    import concourse.mybir as mybir
    import concourse.tile as tile
    import concourse.tile_utils as tile_utils
    from concourse.masks import make_identity

    tile_utils.max_sbuf_usage = 204 * 1024

    F32 = mybir.dt.float32
    F32R = mybir.dt.float32r
    BF16 = mybir.dt.bfloat16
    AF = mybir.ActivationFunctionType
    OP = mybir.AluOpType

    nc = bacc.Bacc(None, target_bir_lowering=False)
    names = {}

    with tile.TileContext(nc) as tc:
        with tc.tile_pool(name="dram", bufs=1, space="DRAM") as dram:
            # ---- external inputs (per-core host-prepared) ----
            xsl_in = dram.tile([TSL, E], F32, kind="ExternalInput")
            wqkv_in = dram.tile([EC, P, 6 * P], BF16, kind="ExternalInput")
            wo_in = dram.tile([H, P, E], BF16, kind="ExternalInput")
            # FFN weights, sequence-parallel: full F dim streamed per core.
            # wgl: [fb=64][p=128][ec=16 x (gate 128 | lin 128)]
            wgl_in = dram.tile([FF // P, P, EC * 2 * P], BF16,
                               kind="ExternalInput")
            # wout: [fs=16][ecol=4][fb=4 x p=128][c=512]
            wout_in = dram.tile([16, 4, 4 * P, 512], BF16,
                                kind="ExternalInput")
            cos_in = dram.tile([TSL, 64], F32, kind="ExternalInput")
            sin_in = dram.tile([TSL, 64], F32, kind="ExternalInput")
            mask_in = dram.tile([4, P, 512], BF16, kind="ExternalInput")
            onec_in = dram.tile([P, 1], F32R, kind="ExternalInput")
            oner_in = dram.tile([1, P], F32R, kind="ExternalInput")
            out_sl = dram.tile([TSL, E], F32, kind="ExternalOutput")
            names.update(
                xsl=xsl_in.name, wqkv=wqkv_in.name, wo=wo_in.name,
                wgl=wgl_in.name, wout=wout_in.name, cos=cos_in.name,
                sin=sin_in.name, mask=mask_in.name, onec=onec_in.name,
                oner=oner_in.name, out=out_sl.name)

            # ---- internal DRAM (collective bounce) ----
            # yT AllGather in 2 token-halves: own [E, 512] -> all [8*E, 512]
            ag_in = [dram.tile([E, 4 * P], BF16, name=f"ag_in{i}")
                     for i in range(2)]
            ag_out = [dram.tile([NCORES * E, 4 * P], BF16, name=f"ag_out{i}",
                                addr_space="Shared")
                      for i in range(2)]
            # AllToAll split in 2 token-halves of each destination slice
            a2a_in = [dram.tile([NCORES * HLOC * P, TSL // 2], BF16,
                                name=f"a2a_in{i}") for i in range(2)]
            a2a_out = [dram.tile([NCORES * HLOC * P, TSL // 2], BF16,
                                 name=f"a2a_out{i}") for i in range(2)]

            RG = [list(range(NCORES))]

            with tc.tile_pool(name="cst", bufs=1) as cst:
                ident = cst.tile([P, P], F32)
                make_identity(nc, ident[:])
                eps_t = cst.tile([P, 1], F32)
                nc.gpsimd.memset(eps_t[:], EPS)

                # residual slice pool opened before stage A (LIFO) so it
                # survives into WO/FFN; DMAs emitted after A0
                x2_stack = ExitStack()
                x2_p = x2_stack.enter_context(tc.tile_pool(name="x2_p", bufs=1))

                # ================= stage A: norm+rope+QKV+attention =========
                stgA = ExitStack()
                wqkv_p = stgA.enter_context(tc.tile_pool(name="wqkv_p", bufs=1))
                tabs = stgA.enter_context(tc.tile_pool(name="tabs", bufs=1))

                wqkv_sb = wqkv_p.tile([P, EC * 6 * P], BF16)
                nc.sync.dma_start(
                    out=wqkv_sb[:].rearrange("p (e c) -> p e c", e=EC),
                    in_=wqkv_in[:].rearrange("e p c -> p e c"))
                ones_col = tabs.tile([P, 1], F32R)
                nc.sync.dma_start(out=ones_col[:], in_=onec_in[:])
                ones_row = tabs.tile([1, P], F32R)
                nc.sync.dma_start(out=ones_row[:], in_=oner_in[:])
                masks = tabs.tile([P, 4 * 512], BF16)
                nc.sync.dma_start(
                    out=masks[:].rearrange("p (m w) -> p m w", m=4),
                    in_=mask_in[:].rearrange("m p w -> p m w"))

                # ---- A0: norm+rope+transpose own slice, then AllGather -----
                a0 = ExitStack()
                a0_sb = a0.enter_context(tc.tile_pool(name="a0_sb", bufs=3))
                a0_ps = a0.enter_context(tc.tile_pool(name="a0_ps", bufs=2, space="PSUM"))

                cos_all = a0_sb.tile([P, 8 * 64], F32, tag="cos")
                sin_all = a0_sb.tile([P, 8 * 64], F32, tag="sin")
                nc.sync.dma_start(
                    out=cos_all[:].rearrange("p (r j) -> p r j", r=8),
                    in_=cos_in[:].rearrange("(r p) j -> p r j", p=P))
                nc.sync.dma_start(
                    out=sin_all[:].rearrange("p (r j) -> p r j", r=8),
                    in_=sin_in[:].rearrange("(r p) j -> p r j", p=P))

                for tt in range(8):
                    x_t = a0_sb.tile([P, E], F32, tag="x")
                    nc.sync.dma_start(out=x_t[:], in_=xsl_in[P * tt:P * (tt + 1), :])
                    scr = a0_sb.tile([P, E], F32, tag="y")
                    ssq = a0_sb.tile([P, 1], F32, tag="ssq")
                    nc.scalar.activation(scr[:], x_t[:], AF.Square,
                                         accum_out=ssq[:])
                    sq = a0_sb.tile([P, 1], F32, tag="sq")
                    nc.scalar.activation(sq[:], ssq[:], AF.Sqrt,
                                         scale=1.0 / E, bias=eps_t[:])
                    s_t = a0_sb.tile([P, 1], F32, tag="s")
                    nc.vector.reciprocal(s_t[:], sq[:])
                    # rope with rmsnorm scale folded in:
                    #   y1 = (x1*s)*cos - (x2*s)*sin
                    #   y2 = (x2*s)*cos + (x1*s)*sin
                    y_t = a0_sb.tile([P, E], F32, tag="y")
                    t1 = a0_sb.tile([P, E], F32, tag="t1")
                    xr = x_t[:].rearrange("p (c two h) -> p c two h", two=2, h=64)
                    yr = y_t[:].rearrange("p (c two h) -> p c two h", two=2, h=64)
                    tr = t1[:].rearrange("p (c two h) -> p c two h", two=2, h=64)
                    cb = cos_all[:, 64 * tt:64 * (tt + 1)].rearrange(
                        "p (o j) -> p o j", o=1).broadcast_to([P, EC, 64])
                    sb_ = sin_all[:, 64 * tt:64 * (tt + 1)].rearrange(
                        "p (o j) -> p o j", o=1).broadcast_to([P, EC, 64])
                    nc.vector.scalar_tensor_tensor(
                        out=tr[:, :, 0], in0=xr[:, :, 1], scalar=s_t[:],
                        in1=sb_, op0=OP.mult, op1=OP.mult)
                    nc.vector.scalar_tensor_tensor(
                        out=yr[:, :, 0], in0=xr[:, :, 0], scalar=s_t[:],
                        in1=cb, op0=OP.mult, op1=OP.mult)
                    nc.vector.tensor_tensor(
                        out=yr[:, :, 0], in0=yr[:, :, 0], in1=tr[:, :, 0],
                        op=OP.subtract)
                    nc.vector.scalar_tensor_tensor(
                        out=tr[:, :, 1], in0=xr[:, :, 0], scalar=s_t[:],
                        in1=sb_, op0=OP.mult, op1=OP.mult)
                    nc.vector.scalar_tensor_tensor(
                        out=yr[:, :, 1], in0=xr[:, :, 1], scalar=s_t[:],
                        in1=cb, op0=OP.mult, op1=OP.mult)
                    nc.vector.tensor_tensor(
                        out=yr[:, :, 1], in0=yr[:, :, 1], in1=tr[:, :, 1],
                        op=OP.add)
                    # transpose 16 chunks, stage out to this token-half's
                    # AllGather input
                    yto = a0_sb.tile([P, EC * P], BF16, tag="yto")
                    yto_v = yto[:].rearrange("p (c t) -> p c t", t=P)
                    for gch in range(4):
                        tps = a0_ps.tile([P, 512], F32, tag="tps")
                        for c4 in range(4):
                            c = 4 * gch + c4
                            nc.tensor.transpose(
                                tps[:, P * c4:P * (c4 + 1)],
                                y_t[:, P * c:P * (c + 1)], ident[:])
                        nc.scalar.copy(
                            yto_v[:, 4 * gch:4 * gch + 4, :],
                            tps[:].rearrange("p (c t) -> p c t", t=P))
                    c = tt // 4
                    nc.sync.dma_start(
                        out=ag_in[c][:].rearrange("(ec p) t -> p ec t", p=P)[
                            :, :, P * (tt % 4):P * (tt % 4 + 1)],
                        in_=yto_v[:, :, :])
                    if tt % 4 == 3:
                        nc.gpsimd.collective_compute(
                            "AllGather", OP.bypass, replica_groups=RG,
                            ins=[ag_in[c][:]], outs=[ag_out[c][:]])
                a0.close()

                # QKV/attention pools open after A0's space is released
                ytb_p = stgA.enter_context(tc.tile_pool(name="ytb", bufs=3))
                qkvb_p = stgA.enter_context(tc.tile_pool(name="qkvb", bufs=2))
                st_sb = stgA.enter_context(tc.tile_pool(name="st_sb", bufs=2))
                at_sb = stgA.enter_context(tc.tile_pool(name="at_sb", bufs=3))
                pr_sb = stgA.enter_context(tc.tile_pool(name="pr_sb", bufs=6))
                st_ps = stgA.enter_context(tc.tile_pool(name="st_ps", bufs=1, space="PSUM"))
                qkv_ps = stgA.enter_context(tc.tile_pool(name="qkv_ps", bufs=3, space="PSUM"))
                at_s_ps = stgA.enter_context(tc.tile_pool(name="at_s_ps", bufs=2, space="PSUM"))
                at_o_ps = stgA.enter_context(tc.tile_pool(name="at_o_ps", bufs=1, space="PSUM"))
                at_db_ps = stgA.enter_context(tc.tile_pool(name="at_db_ps", bufs=1, space="PSUM"))

                # ---- QKV from gathered yT + attention, batch by batch ------
                SC = 1.0 / float(np.sqrt(DH))
                for b in range(B):
                    qt_b = qkvb_p.tile([P, HLOC * S], BF16, tag="qt")
                    kt_b = qkvb_p.tile([P, HLOC * S], BF16, tag="kt")
                    v_b = qkvb_p.tile([P, HLOC * S], BF16, tag="vb")
                    for h in range(2):          # 512-token halves of each rank
                        for rr in (2 * b, 2 * b + 1):
                            ygs = []
                            for ech in range(2):
                                yg = ytb_p.tile([P, 8 * 512], BF16, tag="yg",
                                                name=f"yg{ech}")
                                nc.sync.dma_start(
                                    out=yg[:].rearrange("p (e w) -> p e w",
                                                        e=8),
                                    in_=ag_out[h][
                                        E * rr + 1024 * ech:
                                        E * rr + 1024 * (ech + 1), :
                                    ].rearrange("(e p) t -> p e t", p=P))
                                ygs.append(yg)
                            for g in range(6):
                                pq = qkv_ps.tile([P, 512], F32, tag="pq")
                                for ec in range(EC):
                                    nc.tensor.matmul(
                                        pq[:],
                                        wqkv_sb[:, 6 * P * ec + P * g:
                                                6 * P * ec + P * (g + 1)],
                                        ygs[ec // 8][:, 512 * (ec % 8):
                                                     512 * (ec % 8 + 1)],
                                        start=(ec == 0), stop=(ec == EC - 1))
                                hl = g % 2
                                col = S * hl + 1024 * (rr % 2) + 512 * h
                                if g < 2:      # Q heads, scale by 1/sqrt(DH)
                                    nc.scalar.activation(
                                        qt_b[:, col:col + 512], pq[:], AF.Copy,
                                        scale=SC)
                                elif g < 4:    # K heads
                                    nc.scalar.copy(kt_b[:, col:col + 512], pq[:])
                                else:          # V heads -> transpose to [tok, DH]
                                    vt_tmp = st_sb.tile([P, 512], F32, tag="vt")
                                    nc.scalar.copy(vt_tmp[:], pq[:])
                                    tpv = st_ps.tile([P, 512], F32, tag="tps")
                                    for j in range(4):
                                        nc.tensor.transpose(
                                            tpv[:, P * j:P * (j + 1)],
                                            vt_tmp[:, P * j:P * (j + 1)], ident[:])
                                    nc.scalar.copy(v_b[:, col:col + 512], tpv[:])
                    # ---- attention for batch b, both local heads ----
                    # qb order: evens first so the first-half AllToAll can
                    # fire before the odd qbs of the last batch.
                    for h in range(HLOC):
                        hs = S * h
                        for qb in (0, 2, 1, 3):
                            nk = 4 * qb + 4
                            ps_o = at_o_ps.tile([P, 512], F32, tag="pso")
                            acc = [at_sb.tile([P, 512], F32R, tag="acc0",
                                              name="acc0"),
                                   at_sb.tile([P, 512], F32R, tag="acc1",
                                              name="acc1")]
                            for kt in range(nk):
                                ps_s = at_s_ps.tile([P, 512], F32, tag="pss")
                                nc.tensor.matmul(
                                    ps_s[:],
                                    kt_b[:, hs + P * kt: hs + P * (kt + 1)],
                                    qt_b[:, hs + 512 * qb: hs + 512 * (qb + 1)],
                                    start=True, stop=True)
                                probs = pr_sb.tile([P, 512], BF16, tag="probs")
                                nc.scalar.activation(probs[:], ps_s[:], AF.Exp)
                                if kt >= 4 * qb:
                                    m = kt - 4 * qb
                                    nc.vector.tensor_tensor(
                                        out=probs[:], in0=probs[:],
                                        in1=masks[:, 512 * m:512 * (m + 1)],
                                        op=OP.mult)
                                a_ = acc[kt % 2]
                                eng = nc.gpsimd if kt % 2 == 0 else nc.vector
                                if kt < 2:
                                    eng.tensor_copy(a_[:], probs[:])
                                else:
                                    eng.tensor_tensor(
                                        out=a_[:], in0=a_[:], in1=probs[:],
                                        op=OP.add)
                                nc.tensor.matmul(
                                    ps_o[:],
                                    v_b[:, hs + P * kt: hs + P * (kt + 1)],
                                    probs[:],
                                    start=(kt == 0), stop=(kt == nk - 1),
                                    skip_group_check=True)
                            ps_d = at_db_ps.tile([1, 512], F32, tag="db")
                            nc.tensor.matmul(ps_d[:], ones_col[:], acc[0][:],
                                             start=True, stop=False)
                            nc.tensor.matmul(ps_d[:], ones_col[:], acc[1][:],
                                             start=False, stop=True)
                            rd = at_sb.tile([1, 512], F32R, tag="rd")
                            with nc.allow_low_precision(reason="softmax denom"):
                                nc.vector.reciprocal(rd[:], ps_d[:])
                            ps_b = at_db_ps.tile([P, 512], F32, tag="db")
                            nc.tensor.matmul(ps_b[:], ones_row[:], rd[:],
                                             start=True, stop=True)
                            osb = at_sb.tile([P, 512], F32, tag="osb")
                            nc.scalar.copy(osb[:], ps_o[:])
                            ot = at_sb.tile([P, 512], BF16, tag="ot")
                            nc.vector.tensor_tensor(out=ot[:], in0=osb[:],
                                                    in1=ps_b[:], op=OP.mult)
                            dest = 2 * b + qb // 2
                            a2a_b = a2a_in[qb % 2]
                            a2a_v = a2a_b[:].rearrange("(d r) t -> d r t",
                                                       d=NCORES)
                            nc.sync.dma_start(
                                out=a2a_v[dest, P * h:P * (h + 1), :],
                                in_=ot[:])
                            if b == 3 and h == 1 and qb == 2:
                                # all even-qb outputs written: fire half 0
                                nc.gpsimd.collective_compute(
                                    "AllToAll", OP.bypass, replica_groups=RG,
                                    ins=[a2a_in[0][:]], outs=[a2a_out[0][:]])
                    if b == 0:
                        # residual slice prefetch: emitted once the AllGather
                        # window's DMA burst is over
                        x2_t = [None] * 8
                        for tt in range(8):
                            x2_t[tt] = x2_p.tile([P, E], F32, tag=f"x2_{tt}",
                                                 name=f"x2t{tt}")
                            nc.sync.dma_start(
                                out=x2_t[tt][:],
                                in_=xsl_in[P * tt:P * (tt + 1), :])
                stgA.close()

                nc.gpsimd.collective_compute(
                    "AllToAll", OP.bypass, replica_groups=RG,
                    ins=[a2a_in[1][:]], outs=[a2a_out[1][:]])

                # ================= WO phase (own token slice, 2 halves) =====
                # y2T/norm pools open first so norm2 of half 0 can run while
                # half 1 of WO is still waiting on its AllToAll
                y2T_stack = ExitStack()
                y2T_p = y2T_stack.enter_context(tc.tile_pool(name="y2T_p", bufs=1))
                n2_stack = ExitStack()
                n2_sb = n2_stack.enter_context(tc.tile_pool(name="n2_sb", bufs=2))
                n2_ps = n2_stack.enter_context(tc.tile_pool(name="n2_ps", bufs=2, space="PSUM"))
                y2T_sb = y2T_p.tile([P, EC * TSL], BF16, tag="y2T")

                def norm2_tt(tt):
                    scr2 = n2_sb.tile([P, E], F32, tag="y2", name="scr2")
                    ssq2 = n2_sb.tile([P, 1], F32, tag="ssq2", name="ssq2")
                    nc.scalar.activation(scr2[:], x2_t[tt][:], AF.Square,
                                         accum_out=ssq2[:])
                    sq2 = n2_sb.tile([P, 1], F32, tag="sq2", name="sq2")
                    nc.scalar.activation(sq2[:], ssq2[:], AF.Sqrt,
                                         scale=1.0 / E, bias=eps_t[:])
                    s2 = n2_sb.tile([P, 1], F32, tag="s2", name="s2")
                    nc.vector.reciprocal(s2[:], sq2[:])
                    y2_t = n2_sb.tile([P, E], F32, tag="y2", name="y2t")
                    nc.scalar.activation(y2_t[:], x2_t[tt][:], AF.Copy,
                                         scale=s2[:])
                    y2T_dst = y2T_sb[:].rearrange("p (c t) -> p c t", t=TSL)
                    for gch in range(4):
                        tps = n2_ps.tile([P, 512], F32, tag="tps", name="tps")
                        for c4 in range(4):
                            c = 4 * gch + c4
                            nc.tensor.transpose(
                                tps[:, P * c4:P * (c4 + 1)],
                                y2_t[:, P * c:P * (c + 1)], ident[:])
                        nc.scalar.copy(
                            y2T_dst[:, 4 * gch:4 * gch + 4, P * tt:P * (tt + 1)],
                            tps[:].rearrange("p (c t) -> p c t", t=P))

                wo_pools = ExitStack()
                ot_p = wo_pools.enter_context(tc.tile_pool(name="ot_p", bufs=2))
                wo_p = wo_pools.enter_context(tc.tile_pool(name="wo_p", bufs=2))
                wo_ps = wo_pools.enter_context(tc.tile_pool(name="wo_ps", bufs=2, space="PSUM"))

                for half in range(2):
                    # [2048 hd rows, 512 tok] -> sbuf [p, (hc, tok)]
                    ot_h = ot_p.tile([P, H * 512], BF16, tag="oth")
                    nc.sync.dma_start(
                        out=ot_h[:].rearrange("p (hc t) -> p hc t", hc=H),
                        in_=a2a_out[half][:].rearrange("(hc p) t -> p hc t",
                                                       p=P))
                    for ecol in range(4):
                        wo_c = wo_p.tile([P, H * 512], BF16, tag="woc")
                        nc.sync.dma_start(
                            out=wo_c[:].rearrange("p (hc w) -> p hc w", hc=H),
                            in_=wo_in[:, :, 512 * ecol:512 * (ecol + 1)].rearrange(
                                "hc p w -> p hc w"))
                        for tloc in range(4):
                            tt = 4 * half + tloc
                            ps = wo_ps.tile([P, 512], F32, tag="ps")
                            for hc in range(H):
                                nc.tensor.matmul(
                                    ps[:],
                                    ot_h[:, 512 * hc + P * tloc:
                                         512 * hc + P * (tloc + 1)],
                                    wo_c[:, 512 * hc:512 * (hc + 1)],
                                    start=(hc == 0), stop=(hc == H - 1))
                            nc.vector.tensor_tensor(
                                out=x2_t[tt][:, 512 * ecol:512 * (ecol + 1)],
                                in0=ps[:],
                                in1=x2_t[tt][:, 512 * ecol:512 * (ecol + 1)],
                                op=OP.add)
                    for tloc in range(4):
                        norm2_tt(4 * half + tloc)
                wo_pools.close()
                n2_stack.close()

                # ===== FFN, sequence-parallel: full F dim over own tokens ====
                ffn = ExitStack()
                wgl_p = ffn.enter_context(tc.tile_pool(name="wgl_p", bufs=2))
                wout_p = ffn.enter_context(tc.tile_pool(name="wout_p", bufs=2))
                h_p = ffn.enter_context(tc.tile_pool(name="h_p", bufs=4))
                f1_sb = ffn.enter_context(tc.tile_pool(name="f1_sb", bufs=2))
                f1_ps = ffn.enter_context(tc.tile_pool(name="f1_ps", bufs=1, space="PSUM"))
                f2_ps = ffn.enter_context(tc.tile_pool(name="f2_ps", bufs=2, space="PSUM"))

                NFS = 16                 # f-super blocks of 512
                FBP = 4                  # 128-wide f blocks per super
                for fs in range(NFS):
                    h_t = [None] * FBP
                    for fb in range(FBP):
                        gfb = FBP * fs + fb
                        wgl_sb = wgl_p.tile([P, EC * 2 * P], BF16, tag="wgl")
                        nc.sync.dma_start(out=wgl_sb[:], in_=wgl_in[gfb])
                        ps_g = [None] * 2
                        ps_l = [None] * 2
                        for hf in range(2):
                            ps_g[hf] = f1_ps.tile([P, 512], F32, tag=f"psg{hf}", name=f"psg{hf}")
                            ps_l[hf] = f1_ps.tile([P, 512], F32, tag=f"psl{hf}", name=f"psl{hf}")
                            for ec in range(EC):
                                wb = 2 * P * ec
                                nc.tensor.matmul(
                                    ps_g[hf][:],
                                    wgl_sb[:, wb:wb + P],
                                    y2T_sb[:, TSL * ec + 512 * hf: TSL * ec + 512 * (hf + 1)],
                                    start=(ec == 0), stop=(ec == EC - 1),
                                    skip_group_check=True)
                                nc.tensor.matmul(
                                    ps_l[hf][:],
                                    wgl_sb[:, wb + P:wb + 2 * P],
                                    y2T_sb[:, TSL * ec + 512 * hf: TSL * ec + 512 * (hf + 1)],
                                    start=(ec == 0), stop=(ec == EC - 1),
                                    skip_group_check=True)
                        h_t[fb] = h_p.tile([P, TSL], BF16, tag="h",
                                           name=f"h_{fs}_{fb}")
                        for hf in range(2):
                            tmp_g = f1_sb.tile([P, 512], F32, tag="tmpg")
                            nc.scalar.activation(tmp_g[:], ps_g[hf][:], AF.Gelu)
                            nc.vector.tensor_tensor(
                                out=h_t[fb][:, 512 * hf:512 * (hf + 1)],
                                in0=tmp_g[:], in1=ps_l[hf][:], op=OP.mult)
                    # F2: accumulate this f-super's contribution into x2 acc
                    for ecol in range(4):
                        wout_sb = wout_p.tile([P, FBP * 512], BF16, tag="wout")
                        nc.sync.dma_start(
                            out=wout_sb[:].rearrange("p (b w) -> p b w", b=FBP),
                            in_=wout_in[fs, ecol].rearrange(
                                "(b p) w -> p b w", p=P))
                        for tt in range(8):
                            ps_o = f2_ps.tile([P, 512], F32, tag="pso")
                            for fb in range(FBP):
                                nc.tensor.matmul(
                                    ps_o[:],
                                    h_t[fb][:, P * tt:P * (tt + 1)],
                                    wout_sb[:, 512 * fb:512 * (fb + 1)],
                                    start=(fb == 0), stop=(fb == FBP - 1))
                            nc.vector.tensor_tensor(
                                out=x2_t[tt][:, 512 * ecol:512 * (ecol + 1)],
                                in0=ps_o[:],
                                in1=x2_t[tt][:, 512 * ecol:512 * (ecol + 1)],
                                op=OP.add)
                ffn.close()
                y2T_stack.close()
                for tt in range(8):
                    nc.sync.dma_start(out=out_sl[P * tt:P * (tt + 1), :],
                                      in_=x2_t[tt][:])
                x2_stack.close()
    nc.compile()
    return nc, names


def _prep_shared(inputs):
    """Host-side prep of tensors identical on every core."""
    x = np.ascontiguousarray(
        np.asarray(inputs["inputs"], np.float32).reshape(TOK, E))
    wo = np.asarray(inputs["wo"], np.float32)
    w_gate = np.asarray(inputs["w_gate"], np.float32)
    w_lin = np.asarray(inputs["w_lin"], np.float32)
    w_out = np.asarray(inputs["w_out"], np.float32)
    gamma_attn = np.asarray(inputs["gamma_attn"], np.float32)
    gamma_ffn = np.asarray(inputs["gamma_ffn"], np.float32)
    positions = np.asarray(inputs["positions"])

    wo_h = np.ascontiguousarray(wo.reshape(H, P, E).astype(BF))
    # wgl: [fb=64, p=128, ec=16, gate 128 | lin 128] with gamma_ffn folded
    wg4 = (w_gate * gamma_ffn[:, None]).reshape(EC, P, FF // P, P)
    wl4 = (w_lin * gamma_ffn[:, None]).reshape(EC, P, FF // P, P)
    wgl_h = np.empty((FF // P, P, EC, 2 * P), np.float32)
    wgl_h[..., :P] = wg4.transpose(2, 1, 0, 3)
    wgl_h[..., P:] = wl4.transpose(2, 1, 0, 3)
    wgl_h = np.ascontiguousarray(
        wgl_h.reshape(FF // P, P, EC * 2 * P).astype(BF))
    # wout: [fs=16, ecol=4, fb*p=512, c=512]
    wout_h = np.ascontiguousarray(
        w_out.reshape(16, 4, P, 4, 512).transpose(0, 3, 1, 2, 4)
        .reshape(16, 4, 4 * P, 512).astype(BF))

    # rope tables (gamma_attn folded; reference gamma is all-ones so it is
    # uniform -- assert so silent wrong answers are impossible)
    assert np.all(gamma_attn == gamma_attn[0]), \
        "non-uniform gamma_attn needs full-width rope tables"
    half = DH // 2
    inv_freq = (1.0 / (10000.0 ** (np.arange(half, dtype=np.float32) / half))
                ).astype(np.float32)
    ang = positions.astype(np.float32)[:, None] * inv_freq[None, :]
    g0 = float(gamma_attn[0])
    cos = (np.cos(ang) * g0).astype(np.float32)
    sin = (np.sin(ang) * g0).astype(np.float32)

    k_i = np.arange(P)[:, None]
    q_i = np.arange(512)[None, :]
    msk = np.stack([(P * m + k_i <= q_i) for m in range(4)]).astype(BF)
    return {
        "_x": x, "_cos": cos, "_sin": sin,
        "wo": wo_h, "wgl": wgl_h, "wout": wout_h, "mask": msk,
        "onec": np.ones((P, 1), np.float32),
        "oner": np.ones((1, P), np.float32),
    }


def _prep_inputs(inputs, r, shared):
    """Per-core host-side input prep for core r."""
    wq = np.asarray(inputs["wq"], np.float32)
    wk = np.asarray(inputs["wk"], np.float32)
    wv = np.asarray(inputs["wv"], np.float32)
    h0 = HLOC * r

    def _slice_qkv(w):   # [E, H, DH] -> [EC, P, HLOC*DH]
        return w[:, h0:h0 + HLOC, :].reshape(EC, P, HLOC * DH)

    wqkv = np.concatenate([_slice_qkv(wq), _slice_qkv(wk), _slice_qkv(wv)],
                          axis=2)
    d = {k: v for k, v in shared.items() if not k.startswith("_")}
    d["xsl"] = np.ascontiguousarray(shared["_x"][TSL * r:TSL * (r + 1)])
    sl = slice(1024 * (r % 2), 1024 * (r % 2) + TSL)
    d["cos"] = np.ascontiguousarray(shared["_cos"][sl])
    d["sin"] = np.ascontiguousarray(shared["_sin"][sl])
    d["wqkv"] = np.ascontiguousarray(wqkv.astype(BF))
    return d


def kernel(**inputs) -> np.ndarray:
    from concourse.bass_utils import run_bass_kernel_spmd

    if "nc" not in _CACHE:
        _CACHE["nc"], _CACHE["names"] = _build()
    nc, names = _CACHE["nc"], _CACHE["names"]

    shared = _prep_shared(inputs)
    in_maps = []
    for r in range(NCORES):
        prep = _prep_inputs(inputs, r, shared)
        in_maps.append({names[k]: v for k, v in prep.items()})

    res = run_bass_kernel_spmd(nc, in_maps, core_ids=list(range(NCORES)))
    out = np.empty((TOK, E), np.float32)
    for r in range(NCORES):
        out[TSL * r:TSL * (r + 1)] = res.results[r][names["out"]]
    return out.reshape(B, S, E)

